# revision 1
# baseline (speedup 1.0000x reference)
"""MCSPN Trainium2 kernel: guidance convs + softmax gates + 4-step CSPN recurrence.

Data-parallel over batch: 8 images -> 8 NeuronCores, one image per core.
Per core:
  phase A: conv3x3 (fp32r matmuls, 18 accum MMs/row) -> bias+ReLU (ACT)
           -> conv1x1 (fp32r) -> exp (ACT) -> per-row DMA scatter into
           gate layout e_all [H=128 part, 76*256 free]
  softmax: 3 adds + reciprocal + 4 muls over [128, 19*256] strided views
  phase B: 4 recurrence steps; left/right via guarded 258-wide windows of h,
           up/down via PE shift-matmuls (sub/super-diagonal fp32r matrices)
           into PSUM; gated sums on DVE + GPSIMD.
"""
import os
import sys

sys.path.insert(0, "/opt/trn_rl_repo")

import numpy as np

B, CIN, H, W = 8, 256, 128, 256
K = 19
MID = 128
KD = 4 * K  # 76
EPS = 1e-5
T_STEPS = 4
WP = W + 2  # guarded row width (258)
RG = 8      # feats rows per DMA chunk


def _build():
    import concourse.bacc as bacc
    import concourse.mybir as mybir
    import concourse.tile as tile
    from concourse import bass

    f32 = mybir.dt.float32
    f32r = mybir.dt.float32r
    Act = mybir.ActivationFunctionType
    Alu = mybir.AluOpType

    nc = bacc.Bacc("TRN2", target_bir_lowering=False)

    feats_d = nc.dram_tensor("feats", [CIN, H, W], f32, kind="ExternalInput")
    logits_d = nc.dram_tensor("logits", [K, H, W], f32, kind="ExternalInput")
    w1t_d = nc.dram_tensor("w1t", [128, 2, 9, MID], f32, kind="ExternalInput")
    bmid_d = nc.dram_tensor("bmid", [MID, 1], f32, kind="ExternalInput")
    w2t_d = nc.dram_tensor("w2t", [MID, KD], f32, kind="ExternalInput")
    b2_d = nc.dram_tensor("b2", [KD, 1], f32, kind="ExternalInput")
    sup_d = nc.dram_tensor("sup", [128, 128], f32, kind="ExternalInput")
    sdn_d = nc.dram_tensor("sdn", [128, 128], f32, kind="ExternalInput")
    out_d = nc.dram_tensor("out", [K, H, W], f32, kind="ExternalOutput")

    with tile.TileContext(nc) as tc:
        # ---- long-lived tensors ----
        with tc.tile_pool(name="persist", bufs=1) as pp, \
             tc.tile_pool(name="hpool", bufs=1) as hp:
            e_all = pp.tile([128, KD * W], f32)           # 76 KB/part
            h_a = hp.tile([128, K * WP], f32r)            # 19.6 KB/part
            h_b = hp.tile([128, K * WP], f32r)
            w2_r = pp.tile([MID, KD], f32r)
            bmid = pp.tile([MID, 1], f32)
            b2c = pp.tile([KD, 1], f32)
            s_up = pp.tile([128, 128], f32r)
            s_dn = pp.tile([128, 128], f32r)
            z32 = pp.tile([128, 64], f32)  # zeros source for f32r guard writes

            nc.vector.memset(z32[:], 0.0)
            nc.sync.dma_start(out=bmid[:], in_=bmid_d[:])
            nc.sync.dma_start(out=b2c[:], in_=b2_d[:])
            with tc.tile_pool(name="stage", bufs=1) as stp:
                w2_f = stp.tile([MID, KD], f32)
                s_up_f = stp.tile([128, 128], f32)
                s_dn_f = stp.tile([128, 128], f32)
                nc.sync.dma_start(out=w2_f[:], in_=w2t_d[:])
                nc.vector.tensor_copy(out=w2_r[:], in_=w2_f[:])
                nc.sync.dma_start(out=s_up_f[:], in_=sup_d[:])
                nc.vector.tensor_copy(out=s_up[:], in_=s_up_f[:])
                nc.sync.dma_start(out=s_dn_f[:], in_=sdn_d[:])
                nc.vector.tensor_copy(out=s_dn[:], in_=s_dn_f[:])

            # ================= phase A: guidance =================
            with tc.tile_pool(name="w1p", bufs=1) as w1p:
                w1_f = w1p.tile([128, 2, 9, MID], f32)
                w1_r = w1p.tile([128, 2, 9, MID], f32r)
                nc.sync.dma_start(out=w1_f[:], in_=w1t_d[:])
                nc.vector.tensor_copy(out=w1_r[:], in_=w1_f[:])

                with tc.tile_pool(name="frows", bufs=3) as frp, \
                     tc.tile_pool(name="xrow", bufs=3) as xrp, \
                     tc.tile_pool(name="estrip", bufs=3) as esp, \
                     tc.tile_pool(name="psA", bufs=3, space="PSUM") as psA, \
                     tc.tile_pool(name="psG", bufs=3, space="PSUM") as psG:
                    n_groups = H // RG
                    ftiles = []  # group idx -> tile [128, 2, RG, WP]
                    for gi in range(n_groups):
                        ft = frp.tile([128, 2, RG, WP], f32r, name=f"ft{gi}",
                                      tag="ft")
                        # zero guard columns (both chunks, all rows) via
                        # rounding copy (memset can't write f32r)
                        nc.vector.tensor_copy(
                            out=ft[:, :, :, 0:WP:WP - 1],
                            in_=z32[:, 0:32].rearrange(
                                "p (a b c) -> p a b c", a=2, b=RG))
                        for c in range(2):
                            nc.sync.dma_start(
                                out=ft[:, c, :, 1:W + 1],
                                in_=feats_d[c * 128:(c + 1) * 128,
                                            gi * RG:(gi + 1) * RG, :]
                                .bitcast(f32r))
                        ftiles.append(ft)

                        # process row PAIRS whose input rows (y-1..y+2) are
                        # loaded: N=512 matmuls so LDWEIGHTS hides under the
                        # moving-operand stream.
                        if gi == 0:
                            pairs = [0, 2, 4]
                        elif gi == n_groups - 1:
                            pairs = [8 * gi - 2, 8 * gi, 8 * gi + 2,
                                     8 * gi + 4, 8 * gi + 6]
                        else:
                            pairs = [8 * gi - 2, 8 * gi, 8 * gi + 2, 8 * gi + 4]
                        for y in pairs:
                            acc = psA.tile([MID, 2, W], f32, name="acc")
                            mms = []  # (lhsT_sel, rhs_ap, out_ap)
                            # ky=1 first: always valid + full N=512, so the
                            # start=True matmul covers every PSUM element
                            for ky in (1, 0, 2):
                                for c in range(2):
                                    for kx in range(3):
                                        lw = (c, ky * 3 + kx)
                                        ys, ys2 = y + ky - 1, y + ky
                                        v0 = 0 <= ys < H
                                        v1 = 0 <= ys2 < H
                                        same = (v0 and v1
                                                and ys // RG == ys2 // RG)
                                        if same:
                                            src = ftiles[ys // RG]
                                            mms.append((lw,
                                                src[:, c, ys % RG:ys % RG + 2,
                                                    kx:kx + W],
                                                acc[:, :, :]))
                                        else:
                                            if v0:
                                                src = ftiles[ys // RG]
                                                mms.append((lw,
                                                    src[:, c, ys % RG, kx:kx + W],
                                                    acc[:, 0, :]))
                                            if v1:
                                                src = ftiles[ys2 // RG]
                                                mms.append((lw,
                                                    src[:, c, ys2 % RG, kx:kx + W],
                                                    acc[:, 1, :]))
                            for i, (lw, rhs, oap) in enumerate(mms):
                                nc.tensor.matmul(
                                    out=oap, lhsT=w1_r[:, lw[0], lw[1], :],
                                    rhs=rhs, start=(i == 0),
                                    stop=(i == len(mms) - 1))
                            # relu(x + bias) -> f32r (both rows, FD=512)
                            xr = xrp.tile([MID, 2, W], f32r, name="xr")
                            nc.scalar.activation(xr[:], acc[:], Act.Relu,
                                                 bias=bmid[:], scale=1.0)
                            accg = psG.tile([KD, 2, W], f32, name="accg")
                            nc.tensor.matmul(out=accg[:], lhsT=w2_r[:],
                                             rhs=xr[:], start=True, stop=True)
                            es = esp.tile([KD, 2, W], f32, name="es")
                            nc.scalar.activation(es[:], accg[:], Act.Exp,
                                                 bias=b2c[:], scale=1.0)
                            for r in range(2):
                                nc.sync.dma_start(
                                    out=e_all[y + r:y + r + 1, :].rearrange(
                                        "p (c w) -> p c w", c=KD),
                                    in_=es[:, r, :])

            # ================= softmax over 4 directions =================
            with tc.tile_pool(name="smx", bufs=1) as sp:
                s_all = sp.tile([128, K * W], f32)
                r_all = sp.tile([128, K * W], f32)
                ev = e_all[:].rearrange("p (k d w) -> p k d w", k=K, d=4)
                sv = s_all[:].rearrange("p (k w) -> p k w", k=K)
                nc.vector.tensor_tensor(out=sv, in0=ev[:, :, 0, :],
                                        in1=ev[:, :, 1, :], op=Alu.add)
                nc.vector.tensor_tensor(out=sv, in0=sv,
                                        in1=ev[:, :, 2, :], op=Alu.add)
                nc.vector.tensor_tensor(out=sv, in0=sv,
                                        in1=ev[:, :, 3, :], op=Alu.add)
                rv = r_all[:].rearrange("p (k w) -> p k w", k=K)
                nc.vector.reciprocal(out=r_all[:], in_=s_all[:])
                for d in range(4):
                    eng = nc.vector if d % 2 == 0 else nc.gpsimd
                    eng.tensor_tensor(out=ev[:, :, d, :], in0=ev[:, :, d, :],
                                      in1=rv, op=Alu.mult)

            # ---- load h0 = logits into guarded layout ----
            hv_a = h_a[:].rearrange("p (k w) -> p k w", k=K)
            hv_b = h_b[:].rearrange("p (k w) -> p k w", k=K)
            nc.vector.tensor_copy(
                out=hv_a[:, :, 0:WP:WP - 1],
                in_=z32[:, 0:2 * K].rearrange("p (k g) -> p k g", k=K))
            nc.vector.tensor_copy(
                out=hv_b[:, :, 0:WP:WP - 1],
                in_=z32[:, 0:2 * K].rearrange("p (k g) -> p k g", k=K))
            for k in range(K):
                nc.sync.dma_start(
                    out=h_a[:, k * WP + 1:k * WP + 1 + W],
                    in_=logits_d[k].bitcast(f32r))

            # ================= phase B: recurrence =================
            if True:
                with tc.tile_pool(name="tmp", bufs=4) as tp, \
                     tc.tile_pool(name="psS", bufs=3, space="PSUM") as psS:
                    cur, nxt = h_a, h_b
                    for t in range(T_STEPS):
                        for k in range(K):
                            base = k * WP
                            hwin = cur[:, base:base + WP]
                            up_ps = psS.tile([128, WP], f32, name="up_ps")
                            dn_ps = psS.tile([128, WP], f32, name="dn_ps")
                            nc.tensor.matmul(out=up_ps[:], lhsT=s_up[:],
                                             rhs=hwin, start=True, stop=True)
                            nc.tensor.matmul(out=dn_ps[:], lhsT=s_dn[:],
                                             rhs=hwin, start=True, stop=True)
                            gl = e_all[:, (4 * k + 0) * W:(4 * k + 1) * W]
                            gr = e_all[:, (4 * k + 1) * W:(4 * k + 2) * W]
                            gu = e_all[:, (4 * k + 2) * W:(4 * k + 3) * W]
                            gd = e_all[:, (4 * k + 3) * W:(4 * k + 4) * W]
                            left = cur[:, base:base + W].bitcast(f32)
                            right = cur[:, base + 2:base + 2 + W].bitcast(f32)
                            a = tp.tile([128, W], f32, name="a")
                            b = tp.tile([128, W], f32, name="b")
                            c2 = tp.tile([128, W], f32, name="c2")
                            d2 = tp.tile([128, W], f32, name="d2")
                            nc.vector.tensor_tensor(out=a[:], in0=gl, in1=left,
                                                    op=Alu.mult)
                            nc.gpsimd.tensor_tensor(out=b[:], in0=gr, in1=right,
                                                    op=Alu.mult)
                            nc.vector.tensor_tensor(out=c2[:], in0=gu,
                                                    in1=up_ps[:, 1:W + 1],
                                                    op=Alu.mult)
                            nc.vector.tensor_tensor(out=d2[:], in0=gd,
                                                    in1=dn_ps[:, 1:W + 1],
                                                    op=Alu.mult)
                            nc.gpsimd.tensor_tensor(out=a[:], in0=a[:], in1=b[:],
                                                    op=Alu.add)
                            nc.vector.tensor_tensor(out=c2[:], in0=c2[:],
                                                    in1=d2[:], op=Alu.add)
                            nc.vector.tensor_tensor(
                                out=nxt[:, base + 1:base + 1 + W],
                                in0=a[:], in1=c2[:], op=Alu.add)
                        cur, nxt = nxt, cur

                    for k in range(K):
                        nc.sync.dma_start(
                            out=out_d[k],
                            in_=cur[:, k * WP + 1:k * WP + 1 + W].bitcast(f32))

    nc.compile()
    return nc


_NC_CACHE = None


def kernel(feats, logits, w1, gamma, beta, mean, var, w2, b2):
    global _NC_CACHE
    from concourse.bass_utils import run_bass_kernel_spmd

    feats = np.asarray(feats, dtype=np.float32)
    logits = np.asarray(logits, dtype=np.float32)
    w1 = np.asarray(w1, dtype=np.float32)
    w2 = np.asarray(w2, dtype=np.float32)
    b2 = np.asarray(b2, dtype=np.float32)
    gamma = np.asarray(gamma, dtype=np.float32)
    beta = np.asarray(beta, dtype=np.float32)
    mean = np.asarray(mean, dtype=np.float32)
    var = np.asarray(var, dtype=np.float32)

    inv = gamma / np.sqrt(var + EPS)
    w1f = (w1 * inv[:, None, None, None]).astype(np.float32)  # [MID,CIN,3,3]
    bmid = (beta - mean * inv).astype(np.float32)[:, None]    # [MID,1]
    # [cin_in_chunk 128, chunk 2, tap 9, mid 128]
    w1t = (w1f.transpose(1, 2, 3, 0)                  # [CIN,3,3,MID]
           .reshape(2, 128, 9, MID)
           .transpose(1, 0, 2, 3)).copy()
    w2t = w2.reshape(KD, MID).T.copy()                # [MID,KD]
    b2c = b2[:, None].copy()
    s_up = np.eye(128, k=1, dtype=np.float32)         # out[m]=h[m-1]
    s_dn = np.eye(128, k=-1, dtype=np.float32)        # out[m]=h[m+1]

    if _NC_CACHE is None:
        _NC_CACHE = _build()
    nc = _NC_CACHE

    in_maps = []
    for i in range(B):
        in_maps.append({
            "feats": np.ascontiguousarray(feats[i]),
            "logits": np.ascontiguousarray(logits[i]),
            "w1t": w1t, "bmid": bmid, "w2t": w2t, "b2": b2c,
            "sup": s_up, "sdn": s_dn,
        })

    trace = bool(os.environ.get("KTRACE"))
    res = run_bass_kernel_spmd(nc, in_maps, list(range(B)), trace=trace)
    if trace and res.exec_time_ns is not None:
        print(f"HW exec time: {res.exec_time_ns} ns")
    out = np.stack([res.results[i]["out"] for i in range(B)], axis=0)
    return out.astype(np.float32)


if __name__ == "__main__":
    rng = np.random.default_rng(0)
    ins = {
        "feats": rng.standard_normal((B, CIN, H, W), dtype=np.float32),
        "logits": rng.standard_normal((B, K, H, W), dtype=np.float32),
        "w1": rng.standard_normal((MID, CIN, 3, 3), dtype=np.float32) / 48.0,
        "gamma": rng.standard_normal(MID).astype(np.float32) * 0.1 + 1.0,
        "beta": rng.standard_normal(MID).astype(np.float32) * 0.1,
        "mean": rng.standard_normal(MID).astype(np.float32) * 0.1,
        "var": rng.random(MID).astype(np.float32) + 0.5,
        "w2": rng.standard_normal((KD, MID, 1, 1)).astype(np.float32) / 11.3,
        "b2": rng.standard_normal(KD).astype(np.float32) * 0.01,
    }
    o = kernel(**ins)
    print("kernel out", o.shape, o.dtype, np.abs(o).mean())



# revision 3
# speedup vs baseline: 1.1693x; 1.1693x over previous
"""MCSPN Trainium2 kernel: guidance convs + softmax gates + 4-step CSPN recurrence.

Data-parallel over batch: 8 images -> 8 NeuronCores, one image per core.
Per core:
  phase A: conv3x3 in bf16 (18 accum MMs/row-pair, per-tap column windows so
           feats tiles need no guard columns -> contiguous 8KB DMA packets)
           -> bias+ReLU (ACT, bf16 out) -> conv1x1 bf16 -> exp (ACT)
           -> per-row DMA scatter into d-major gate layout
           e_all [H=128 part, 4dir, 19k, 256w] (f32)
  softmax: 2 adds (DVE+GPSIMD) + reciprocal_approx_fast + 4 muls over
           contiguous [128, 4864] views; boundary gate cols zeroed so the
           recurrence can use a flat h layout.
  phase B: h stored flat-guarded [128, 1+19*256+1] f32r; left/right via AP
           offsets (cross-k bleed killed by zeroed gates), up/down via PE
           shift-matmuls into PSUM k-chunks; 7 elementwise ops per step as
           full-K contiguous tensors split across DVE/GPSIMD.
"""
import os
import sys

sys.path.insert(0, "/opt/trn_rl_repo")

import numpy as np

B, CIN, H, W = 8, 256, 128, 256
K = 19
MID = 128
KD = 4 * K  # 76
EPS = 1e-5
T_STEPS = 4
RG = 16          # feats rows per DMA group
NG = H // RG     # 8
KW = K * W       # 4864
FH = KW + 2      # flat guarded h width


def _build():
    import concourse.bacc as bacc
    import concourse.mybir as mybir
    import concourse.tile as tile

    f32 = mybir.dt.float32
    f32r = mybir.dt.float32r
    bf16 = mybir.dt.bfloat16
    Act = mybir.ActivationFunctionType
    Alu = mybir.AluOpType

    nc = bacc.Bacc("TRN2", target_bir_lowering=False)

    feats_d = nc.dram_tensor("feats", [CIN, H, W], bf16, kind="ExternalInput")
    logits_d = nc.dram_tensor("logits", [K, H, W], f32, kind="ExternalInput")
    w1t_d = nc.dram_tensor("w1t", [128, 2, 9, MID], bf16, kind="ExternalInput")
    bmid_d = nc.dram_tensor("bmid", [MID, 1], f32, kind="ExternalInput")
    w2t_d = nc.dram_tensor("w2t", [MID, KD], bf16, kind="ExternalInput")
    b2_d = nc.dram_tensor("b2", [KD, 1], f32, kind="ExternalInput")
    sup_d = nc.dram_tensor("sup", [128, 128], f32, kind="ExternalInput")
    sdn_d = nc.dram_tensor("sdn", [128, 128], f32, kind="ExternalInput")
    out_d = nc.dram_tensor("out", [K, H, W], f32, kind="ExternalOutput")

    # kx -> (ic0, ic1, oc0, oc1): out[:, oc0:oc1] += w[kx].T @ in[:, ic0:ic1]
    WIN = {0: (0, W - 1, 1, W), 1: (0, W, 0, W), 2: (1, W, 0, W - 1)}

    with tile.TileContext(nc) as tc:
        with tc.tile_pool(name="persist", bufs=1) as pp, \
             tc.tile_pool(name="hpool", bufs=1) as hp:
            e_all = pp.tile([128, 4, K, W], f32)   # d-major gates, 76KB/part
            h_a = hp.tile([128, FH], f32r)
            h_b = hp.tile([128, FH], f32r)
            w1_r = pp.tile([128, 2, 9, MID], bf16)
            w2_r = pp.tile([MID, KD], bf16)
            bmid = pp.tile([MID, 1], f32)
            b2c = pp.tile([KD, 1], f32)
            s_up = pp.tile([128, 128], f32r)
            s_dn = pp.tile([128, 128], f32r)
            z2 = pp.tile([128, 2], f32)

            nc.vector.memset(z2[:], 0.0)
            # zero guard columns (0 and FH-1) of both h buffers
            nc.vector.tensor_copy(out=h_a[:, 0:FH:FH - 1], in_=z2[:])
            nc.vector.tensor_copy(out=h_b[:, 0:FH:FH - 1], in_=z2[:])
            # h0 = logits, issued first so it overlaps all of phase A
            for k in range(K):
                eng = nc.sync if k % 2 == 0 else nc.scalar
                eng.dma_start(out=h_a[:, 1 + k * W:1 + (k + 1) * W],
                              in_=logits_d[k].bitcast(f32r))
            nc.sync.dma_start(out=w1_r[:], in_=w1t_d[:])
            nc.sync.dma_start(out=w2_r[:], in_=w2t_d[:])
            nc.sync.dma_start(out=bmid[:], in_=bmid_d[:])
            nc.sync.dma_start(out=b2c[:], in_=b2_d[:])
            with tc.tile_pool(name="stage", bufs=1) as stp:
                s_up_f = stp.tile([128, 128], f32)
                s_dn_f = stp.tile([128, 128], f32)
                nc.scalar.dma_start(out=s_up_f[:], in_=sup_d[:])
                nc.vector.tensor_copy(out=s_up[:], in_=s_up_f[:])
                nc.scalar.dma_start(out=s_dn_f[:], in_=sdn_d[:])
                nc.vector.tensor_copy(out=s_dn[:], in_=s_dn_f[:])

            # ================= phase A: guidance =================
            with tc.tile_pool(name="frows", bufs=4) as frp, \
                 tc.tile_pool(name="xrow", bufs=3) as xrp, \
                 tc.tile_pool(name="estrip", bufs=3) as esp, \
                 tc.tile_pool(name="psA", bufs=4, space="PSUM") as psA, \
                 tc.tile_pool(name="psG", bufs=4, space="PSUM") as psG:
                ftiles = {}

                def load_group(gi):
                    ft = frp.tile([128, 2, RG, W], bf16, name=f"ft{gi}",
                                  tag="ft")
                    for c in range(2):
                        eng = nc.sync if c == 0 else nc.scalar
                        eng.dma_start(
                            out=ft[:, c],
                            in_=feats_d[c * 128:(c + 1) * 128,
                                        gi * RG:(gi + 1) * RG, :])
                    ftiles[gi] = ft

                load_group(0)
                load_group(1)
                for g in range(NG):
                    if g + 2 < NG:
                        load_group(g + 2)
                    for y in range(RG * g, RG * g + RG - 1, 2):
                        acc = psA.tile([MID, 2, W], f32, name="acc")
                        mms = []  # (c, tap, rhs_ap, out_ap)
                        # ky=1 first with kx=1 first: full N=512 start matmul
                        for ky in (1, 0, 2):
                            r0 = y + ky - 1
                            r1 = r0 + 1
                            v0 = 0 <= r0 < H
                            v1 = 0 <= r1 < H
                            same = v0 and v1 and (r0 // RG == r1 // RG)
                            kxs = (1, 0, 2) if ky == 1 else (0, 1, 2)
                            for c in range(2):
                                for kx in kxs:
                                    ic0, ic1, oc0, oc1 = WIN[kx]
                                    tap = ky * 3 + kx
                                    if same:
                                        src = ftiles[r0 // RG]
                                        mms.append((c, tap,
                                            src[:, c, r0 % RG:r0 % RG + 2,
                                                ic0:ic1],
                                            acc[:, 0:2, oc0:oc1]))
                                    else:
                                        if v0:
                                            mms.append((c, tap,
                                                ftiles[r0 // RG][:, c, r0 % RG,
                                                                 ic0:ic1],
                                                acc[:, 0, oc0:oc1]))
                                        if v1:
                                            mms.append((c, tap,
                                                ftiles[r1 // RG][:, c, r1 % RG,
                                                                 ic0:ic1],
                                                acc[:, 1, oc0:oc1]))
                        for i, (c, tap, rhs, oap) in enumerate(mms):
                            nc.tensor.matmul(out=oap,
                                             lhsT=w1_r[:, c, tap, :],
                                             rhs=rhs, start=(i == 0),
                                             stop=(i == len(mms) - 1))
                        xr = xrp.tile([MID, 2, W], bf16, name="xr")
                        nc.scalar.activation(xr[:], acc[:], Act.Relu,
                                             bias=bmid[:], scale=1.0)
                        accg = psG.tile([KD, 2, W], f32, name="accg")
                        nc.tensor.matmul(out=accg[:], lhsT=w2_r[:],
                                         rhs=xr[:], start=True, stop=True)
                        es = esp.tile([KD, 2, W], f32, name="es")
                        nc.scalar.activation(es[:], accg[:], Act.Exp,
                                             bias=b2c[:], scale=1.0)
                        for r in range(2):
                            eng = nc.sync if (y // 2 + r) % 2 == 0 else nc.scalar
                            eng.dma_start(out=e_all[y + r:y + r + 1],
                                          in_=es[:, r, :])

            # ================= softmax over 4 directions =================
            with tc.tile_pool(name="smx", bufs=1) as sp:
                s_all = sp.tile([128, KW], f32)
                r_all = sp.tile([128, KW], f32)
                s3 = s_all[:].rearrange("p (k w) -> p k w", k=K)
                r3 = r_all[:].rearrange("p (k w) -> p k w", k=K)
                nc.vector.tensor_tensor(out=s3, in0=e_all[:, 0],
                                        in1=e_all[:, 1], op=Alu.add)
                nc.gpsimd.tensor_tensor(out=r3, in0=e_all[:, 2],
                                        in1=e_all[:, 3], op=Alu.add)
                nc.vector.tensor_tensor(out=s_all[:], in0=s_all[:],
                                        in1=r_all[:], op=Alu.add)
                nc.vector.reciprocal_approx_fast(out=r_all[:], in_=s_all[:])
                for d in range(4):
                    eng = nc.vector if d in (0, 2) else nc.gpsimd
                    eng.tensor_tensor(out=e_all[:, d], in0=e_all[:, d],
                                      in1=r3, op=Alu.mult)
                # zero boundary gates so flat-h cross-k reads contribute 0
                nc.vector.memset(e_all[:, 0, :, 0:1], 0.0)
                nc.gpsimd.memset(e_all[:, 1, :, W - 1:W], 0.0)

            # ================= phase B: recurrence =================
            KCH = [(0, 4), (4, 4), (8, 4), (12, 4), (16, 3)]
            with tc.tile_pool(name="pbt", bufs=1) as tp, \
                 tc.tile_pool(name="psS", bufs=2, space="PSUM") as psS:
                af = tp.tile([128, K, W], f32)
                bf = tp.tile([128, K, W], f32)
                cf = tp.tile([128, K, W], f32)
                df = tp.tile([128, K, W], f32)
                cur, nxt = h_a, h_b
                for t in range(T_STEPS):
                    lv = cur[:, 0:KW].bitcast(f32).rearrange(
                        "p (k w) -> p k w", k=K)
                    rv = cur[:, 2:2 + KW].bitcast(f32).rearrange(
                        "p (k w) -> p k w", k=K)
                    nc.vector.tensor_tensor(out=af[:], in0=e_all[:, 0],
                                            in1=lv, op=Alu.mult)
                    nc.gpsimd.tensor_tensor(out=bf[:], in0=e_all[:, 1],
                                            in1=rv, op=Alu.mult)
                    nc.gpsimd.tensor_tensor(out=af[:], in0=af[:], in1=bf[:],
                                            op=Alu.add)
                    for (k0, nk) in KCH:
                        ps = psS.tile([128, 2, 4, W], f32, name="ps")
                        for j in range(nk):
                            nc.tensor.matmul(
                                out=ps[:, 0, j], lhsT=s_up[:],
                                rhs=cur[:, 1 + (k0 + j) * W:
                                        1 + (k0 + j + 1) * W],
                                start=True, stop=True)
                        for j in range(nk):
                            nc.tensor.matmul(
                                out=ps[:, 1, j], lhsT=s_dn[:],
                                rhs=cur[:, 1 + (k0 + j) * W:
                                        1 + (k0 + j + 1) * W],
                                start=True, stop=True)
                        nc.vector.tensor_tensor(out=cf[:, k0:k0 + nk],
                                                in0=e_all[:, 2, k0:k0 + nk],
                                                in1=ps[:, 0, 0:nk],
                                                op=Alu.mult)
                        nc.vector.tensor_tensor(out=df[:, k0:k0 + nk],
                                                in0=e_all[:, 3, k0:k0 + nk],
                                                in1=ps[:, 1, 0:nk],
                                                op=Alu.mult)
                    nc.gpsimd.tensor_tensor(out=cf[:], in0=cf[:], in1=df[:],
                                            op=Alu.add)
                    # out AP stays f32r so DVE does the rounding write the
                    # f32r matmuls require
                    ov = nxt[:, 1:1 + KW].rearrange("p (k w) -> p k w", k=K)
                    nc.vector.tensor_tensor(out=ov, in0=af[:], in1=cf[:],
                                            op=Alu.add)
                    if t == T_STEPS - 1:
                        for k in range(K):
                            eng = nc.sync if k % 2 == 0 else nc.scalar
                            eng.dma_start(
                                out=out_d[k],
                                in_=nxt[:, 1 + k * W:
                                        1 + (k + 1) * W].bitcast(f32))
                    cur, nxt = nxt, cur

    nc.compile()
    return nc


_NC_CACHE = None


def kernel(feats, logits, w1, gamma, beta, mean, var, w2, b2):
    global _NC_CACHE
    from concourse.bass_utils import run_bass_kernel_spmd
    from ml_dtypes import bfloat16

    feats = np.asarray(feats, dtype=np.float32)
    logits = np.asarray(logits, dtype=np.float32)
    w1 = np.asarray(w1, dtype=np.float32)
    w2 = np.asarray(w2, dtype=np.float32)
    b2 = np.asarray(b2, dtype=np.float32)
    gamma = np.asarray(gamma, dtype=np.float32)
    beta = np.asarray(beta, dtype=np.float32)
    mean = np.asarray(mean, dtype=np.float32)
    var = np.asarray(var, dtype=np.float32)

    inv = gamma / np.sqrt(var + EPS)
    w1f = (w1 * inv[:, None, None, None]).astype(np.float32)  # [MID,CIN,3,3]
    bmid = (beta - mean * inv).astype(np.float32)[:, None]    # [MID,1]
    # [cin_in_chunk 128, chunk 2, tap 9, mid 128], bf16
    w1t = (w1f.transpose(1, 2, 3, 0)                  # [CIN,3,3,MID]
           .reshape(2, 128, 9, MID)
           .transpose(1, 0, 2, 3)).astype(bfloat16)
    # d-major channel order: new channel j = d*19+k  <-  old channel k*4+d
    perm = np.array([4 * (j % K) + (j // K) for j in range(KD)])
    w2m = w2.reshape(KD, MID)[perm]
    w2t = np.ascontiguousarray(w2m.T).astype(bfloat16)  # [MID,KD]
    b2c = b2[perm][:, None].astype(np.float32)
    s_up = np.eye(128, k=1, dtype=np.float32)         # out[m]=h[m-1]
    s_dn = np.eye(128, k=-1, dtype=np.float32)        # out[m]=h[m+1]
    feats_bf = feats.astype(bfloat16)

    if _NC_CACHE is None:
        _NC_CACHE = _build()
    nc = _NC_CACHE

    in_maps = []
    for i in range(B):
        in_maps.append({
            "feats": np.ascontiguousarray(feats_bf[i]),
            "logits": np.ascontiguousarray(logits[i]),
            "w1t": w1t, "bmid": bmid, "w2t": w2t, "b2": b2c,
            "sup": s_up, "sdn": s_dn,
        })

    trace = bool(os.environ.get("KTRACE"))
    res = run_bass_kernel_spmd(nc, in_maps, list(range(B)), trace=trace)
    if trace and res.exec_time_ns is not None:
        print(f"HW exec time: {res.exec_time_ns} ns")
    out = np.stack([res.results[i]["out"] for i in range(B)], axis=0)
    return out.astype(np.float32)


if __name__ == "__main__":
    rng = np.random.default_rng(0)
    ins = {
        "feats": rng.standard_normal((B, CIN, H, W), dtype=np.float32),
        "logits": rng.standard_normal((B, K, H, W), dtype=np.float32),
        "w1": rng.standard_normal((MID, CIN, 3, 3), dtype=np.float32) / 48.0,
        "gamma": rng.standard_normal(MID).astype(np.float32) * 0.1 + 1.0,
        "beta": rng.standard_normal(MID).astype(np.float32) * 0.1,
        "mean": rng.standard_normal(MID).astype(np.float32) * 0.1,
        "var": rng.random(MID).astype(np.float32) + 0.5,
        "w2": rng.standard_normal((KD, MID, 1, 1)).astype(np.float32) / 11.3,
        "b2": rng.standard_normal(KD).astype(np.float32) * 0.01,
    }
    o = kernel(**ins)
    print("kernel out", o.shape, o.dtype, np.abs(o).mean())


# revision 5
# speedup vs baseline: 1.5308x; 1.3092x over previous
"""MCSPN Trainium2 kernel: guidance convs + softmax gates + 4-step CSPN recurrence.

Data-parallel over batch: 8 images -> 8 NeuronCores, one image per core.
Per core:
  phase A: conv3x3 in bf16 (18 accum MMs/row-pair, per-tap column windows so
           feats tiles need no guard columns -> contiguous 8KB DMA packets)
           -> bias+ReLU (ACT, bf16 out) -> conv1x1 bf16 -> exp (ACT, bf16)
           -> per-row DMA scatter into d-major gate layout
           e_all [H=128 part, 4dir, 19k, 256w] (bf16)
  softmax: adds -> reciprocal_approx_fast -> 4 normalize muls (bf16 gates);
           boundary gate cols zeroed so the recurrence can use a flat h
           layout; up/dn gates pre-shifted by one row (PE matmul) so the
           recurrence's gating happens BEFORE the shift matmul.
  phase B: h flat-guarded [128, 1+19*256+1] bf16. Per step: a=g0*left,
           b=g1*right via AP offsets; u=g2'*h, v=g3'*h; up+dn contributions
           = s_up@u + s_dn@v ACCUMULATED in PSUM by the shift matmuls;
           fin = (a+b) + psum per k-chunk. 5 DVE-size ops + 1 GPSIMD op
           per step, all full-K contiguous.
"""
import os
import sys

sys.path.insert(0, "/opt/trn_rl_repo")

import numpy as np

B, CIN, H, W = 8, 256, 128, 256
K = 19
MID = 128
KD = 4 * K  # 76
EPS = 1e-5
T_STEPS = 4
RG = 16          # feats rows per DMA group
NG = H // RG     # 8
KW = K * W       # 4864
FH = KW + 2      # flat guarded h width
PCH = [(0, 8), (8, 8), (16, 3)]  # k-chunks for PSUM-bound work


def _build():
    import concourse.bacc as bacc
    import concourse.mybir as mybir
    import concourse.tile as tile

    f32 = mybir.dt.float32
    bf16 = mybir.dt.bfloat16
    Act = mybir.ActivationFunctionType
    Alu = mybir.AluOpType

    nc = bacc.Bacc("TRN2", target_bir_lowering=False)

    feats_d = nc.dram_tensor("feats", [CIN, H, W], bf16, kind="ExternalInput")
    logits_d = nc.dram_tensor("logits", [K, H, W], bf16, kind="ExternalInput")
    w1t_d = nc.dram_tensor("w1t", [128, 2, 9, MID], bf16, kind="ExternalInput")
    bmid_d = nc.dram_tensor("bmid", [MID, 1], f32, kind="ExternalInput")
    w2t_d = nc.dram_tensor("w2t", [MID, KD], bf16, kind="ExternalInput")
    b2_d = nc.dram_tensor("b2", [KD, 1], f32, kind="ExternalInput")
    sup_d = nc.dram_tensor("sup", [128, 128], bf16, kind="ExternalInput")
    sdn_d = nc.dram_tensor("sdn", [128, 128], bf16, kind="ExternalInput")
    out_d = nc.dram_tensor("out", [K, H, W], f32, kind="ExternalOutput")

    # kx -> (ic0, ic1, oc0, oc1): out[:, oc0:oc1] += w[kx].T @ in[:, ic0:ic1]
    WIN = {0: (0, W - 1, 1, W), 1: (0, W, 0, W), 2: (1, W, 0, W - 1)}

    with tile.TileContext(nc) as tc:
        with tc.tile_pool(name="persist", bufs=1) as pp, \
             tc.tile_pool(name="hpool", bufs=1) as hp:
            e_all = pp.tile([128, 4, K, W], bf16)  # d-major gates, 38KB/part
            h_a = hp.tile([128, FH], bf16)
            h_b = hp.tile([128, FH], bf16)
            w1_r = pp.tile([128, 2, 9, MID], bf16)
            w2_r = pp.tile([MID, KD], bf16)
            bmid = pp.tile([MID, 1], f32)
            b2c = pp.tile([KD, 1], f32)
            s_up = pp.tile([128, 128], bf16)
            s_dn = pp.tile([128, 128], bf16)

            # zero guard columns (0 and FH-1) of both h buffers
            nc.vector.memset(h_a[:, 0:FH:FH - 1], 0.0)
            nc.vector.memset(h_b[:, 0:FH:FH - 1], 0.0)
            # h0 = logits, issued first so it overlaps all of phase A
            for k in range(K):
                nc.sync.dma_start(out=h_a[:, 1 + k * W:1 + (k + 1) * W],
                                  in_=logits_d[k])
            nc.sync.dma_start(out=w1_r[:], in_=w1t_d[:])
            nc.sync.dma_start(out=w2_r[:], in_=w2t_d[:])
            nc.sync.dma_start(out=bmid[:], in_=bmid_d[:])
            nc.sync.dma_start(out=b2c[:], in_=b2_d[:])
            nc.sync.dma_start(out=s_up[:], in_=sup_d[:])
            nc.sync.dma_start(out=s_dn[:], in_=sdn_d[:])

            # ================= phase A: guidance =================
            with tc.tile_pool(name="frows", bufs=4) as frp, \
                 tc.tile_pool(name="xrow", bufs=3) as xrp, \
                 tc.tile_pool(name="estrip", bufs=3) as esp, \
                 tc.tile_pool(name="psA", bufs=5, space="PSUM") as psA, \
                 tc.tile_pool(name="psG", bufs=3, space="PSUM") as psG:
                ftiles = {}

                def load_group(gi):
                    ft = frp.tile([128, 2, RG, W], bf16, name=f"ft{gi}",
                                  tag="ft")
                    for c in range(2):
                        nc.sync.dma_start(
                            out=ft[:, c],
                            in_=feats_d[c * 128:(c + 1) * 128,
                                        gi * RG:(gi + 1) * RG, :])
                    ftiles[gi] = ft

                load_group(0)
                load_group(1)
                for g in range(NG):
                    if g + 2 < NG:
                        load_group(g + 2)
                    for y in range(RG * g, RG * g + RG - 1, 2):
                        acc = psA.tile([MID, 2, W], f32, name="acc")
                        mms = []  # (c, tap, rhs_ap, out_ap)
                        # ky=1 first with kx=1 first: full N=512 start matmul
                        for ky in (1, 0, 2):
                            r0 = y + ky - 1
                            r1 = r0 + 1
                            v0 = 0 <= r0 < H
                            v1 = 0 <= r1 < H
                            same = v0 and v1 and (r0 // RG == r1 // RG)
                            kxs = (1, 0, 2) if ky == 1 else (0, 1, 2)
                            for c in range(2):
                                for kx in kxs:
                                    ic0, ic1, oc0, oc1 = WIN[kx]
                                    tap = ky * 3 + kx
                                    if same:
                                        src = ftiles[r0 // RG]
                                        mms.append((c, tap,
                                            src[:, c, r0 % RG:r0 % RG + 2,
                                                ic0:ic1],
                                            acc[:, 0:2, oc0:oc1]))
                                    else:
                                        if v0:
                                            mms.append((c, tap,
                                                ftiles[r0 // RG][:, c, r0 % RG,
                                                                 ic0:ic1],
                                                acc[:, 0, oc0:oc1]))
                                        if v1:
                                            mms.append((c, tap,
                                                ftiles[r1 // RG][:, c, r1 % RG,
                                                                 ic0:ic1],
                                                acc[:, 1, oc0:oc1]))
                        for i, (c, tap, rhs, oap) in enumerate(mms):
                            nc.tensor.matmul(out=oap,
                                             lhsT=w1_r[:, c, tap, :],
                                             rhs=rhs, start=(i == 0),
                                             stop=(i == len(mms) - 1))
                        xr = xrp.tile([MID, 2, W], bf16, name="xr")
                        nc.scalar.activation(xr[:], acc[:], Act.Relu,
                                             bias=bmid[:], scale=1.0)
                        accg = psG.tile([KD, 2, W], f32, name="accg")
                        nc.tensor.matmul(out=accg[:], lhsT=w2_r[:],
                                         rhs=xr[:], start=True, stop=True)
                        es = esp.tile([KD, 2, W], bf16, name="es")
                        nc.scalar.activation(es[:], accg[:], Act.Exp,
                                             bias=b2c[:], scale=1.0)
                        for r in range(2):
                            nc.sync.dma_start(out=e_all[y + r:y + r + 1],
                                              in_=es[:, r, :])

            # ============ softmax + gate pre-shift ============
            # g2' = s_dn @ g2 (g2'[p] = g2[p+1]), g3' = s_up @ g3, so the
            # recurrence can gate BEFORE the shift matmul.
            g2p = pp.tile([128, K, W], bf16)
            g3p = pp.tile([128, K, W], bf16)
            with tc.tile_pool(name="smx", bufs=1) as sp, \
                 tc.tile_pool(name="psSM", bufs=2, space="PSUM") as psSM:
                s_all = sp.tile([128, KW], f32)
                t_all = sp.tile([128, KW], f32)
                s3 = s_all[:].rearrange("p (k w) -> p k w", k=K)
                t3 = t_all[:].rearrange("p (k w) -> p k w", k=K)
                nc.vector.tensor_tensor(out=s3, in0=e_all[:, 0],
                                        in1=e_all[:, 1], op=Alu.add)
                nc.vector.tensor_tensor(out=t3, in0=e_all[:, 2],
                                        in1=e_all[:, 3], op=Alu.add)
                nc.vector.tensor_tensor(out=s_all[:], in0=s_all[:],
                                        in1=t_all[:], op=Alu.add)
                # reuse t_all as the reciprocal
                nc.vector.reciprocal_approx_fast(out=t_all[:], in_=s_all[:])
                for d in range(4):
                    nc.vector.tensor_tensor(out=e_all[:, d], in0=e_all[:, d],
                                            in1=t3, op=Alu.mult)
                # zero boundary gates so flat-h cross-k reads contribute 0
                nc.vector.memset(e_all[:, 0, :, 0:1], 0.0)
                nc.vector.memset(e_all[:, 1, :, W - 1:W], 0.0)
                for (k0, nk) in PCH:
                    for (gp, mat, d) in ((g2p, s_dn, 2), (g3p, s_up, 3)):
                        psg = psSM.tile([128, 8, W], f32, name="psg")
                        for j in range(0, nk - 1, 2):
                            nc.tensor.matmul(
                                out=psg[:, j:j + 2],
                                lhsT=mat[:],
                                rhs=e_all[:, d, k0 + j:k0 + j + 2],
                                start=True, stop=True)
                        if nk % 2:
                            nc.tensor.matmul(
                                out=psg[:, nk - 1], lhsT=mat[:],
                                rhs=e_all[:, d, k0 + nk - 1],
                                start=True, stop=True)
                        nc.vector.tensor_copy(out=gp[:, k0:k0 + nk],
                                              in_=psg[:, 0:nk])

            # ================= phase B: recurrence =================
            with tc.tile_pool(name="pbt", bufs=1) as tp, \
                 tc.tile_pool(name="psS", bufs=2, space="PSUM") as psS:
                af = tp.tile([128, K, W], bf16)
                bf = tp.tile([128, K, W], bf16)
                ab = tp.tile([128, K, W], bf16)
                uf = tp.tile([128, K, W], bf16)
                vf = tp.tile([128, K, W], bf16)
                o32 = tp.tile([128, K, W], f32)
                cur, nxt = h_a, h_b
                for t in range(T_STEPS):
                    cv = cur[:, 1:1 + KW].rearrange("p (k w) -> p k w", k=K)
                    lv = cur[:, 0:KW].rearrange("p (k w) -> p k w", k=K)
                    rv = cur[:, 2:2 + KW].rearrange("p (k w) -> p k w", k=K)
                    nc.vector.tensor_tensor(out=uf[:], in0=g2p[:], in1=cv,
                                            op=Alu.mult)
                    nc.vector.tensor_tensor(out=vf[:], in0=g3p[:], in1=cv,
                                            op=Alu.mult)
                    nc.vector.tensor_tensor(out=af[:], in0=e_all[:, 0],
                                            in1=lv, op=Alu.mult)
                    nc.vector.tensor_tensor(out=bf[:], in0=e_all[:, 1],
                                            in1=rv, op=Alu.mult)
                    nc.gpsimd.tensor_tensor(out=ab[:], in0=af[:], in1=bf[:],
                                            op=Alu.add)
                    last = t == T_STEPS - 1
                    for (k0, nk) in PCH:
                        ps = psS.tile([128, 8, W], f32, name="ps")
                        for j in range(nk):
                            nc.tensor.matmul(out=ps[:, j], lhsT=s_up[:],
                                             rhs=uf[:, k0 + j],
                                             start=True, stop=False)
                            nc.tensor.matmul(out=ps[:, j], lhsT=s_dn[:],
                                             rhs=vf[:, k0 + j],
                                             start=False, stop=True)
                        if last:
                            oslice = o32[:, k0:k0 + nk]
                        else:
                            oslice = nxt[:, 1 + k0 * W:
                                         1 + (k0 + nk) * W].rearrange(
                                             "p (k w) -> p k w", k=nk)
                        nc.vector.tensor_tensor(out=oslice,
                                                in0=ab[:, k0:k0 + nk],
                                                in1=ps[:, 0:nk], op=Alu.add)
                        if last:
                            for k in range(k0, k0 + nk):
                                nc.sync.dma_start(out=out_d[k],
                                                  in_=o32[:, k])
                    cur, nxt = nxt, cur

    nc.compile()
    return nc


_NC_CACHE = None


def kernel(feats, logits, w1, gamma, beta, mean, var, w2, b2):
    global _NC_CACHE
    from concourse.bass_utils import run_bass_kernel_spmd
    from ml_dtypes import bfloat16

    feats = np.asarray(feats, dtype=np.float32)
    logits = np.asarray(logits, dtype=np.float32)
    w1 = np.asarray(w1, dtype=np.float32)
    w2 = np.asarray(w2, dtype=np.float32)
    b2 = np.asarray(b2, dtype=np.float32)
    gamma = np.asarray(gamma, dtype=np.float32)
    beta = np.asarray(beta, dtype=np.float32)
    mean = np.asarray(mean, dtype=np.float32)
    var = np.asarray(var, dtype=np.float32)

    inv = gamma / np.sqrt(var + EPS)
    w1f = (w1 * inv[:, None, None, None]).astype(np.float32)  # [MID,CIN,3,3]
    bmid = (beta - mean * inv).astype(np.float32)[:, None]    # [MID,1]
    # [cin_in_chunk 128, chunk 2, tap 9, mid 128], bf16
    w1t = (w1f.transpose(1, 2, 3, 0)                  # [CIN,3,3,MID]
           .reshape(2, 128, 9, MID)
           .transpose(1, 0, 2, 3)).astype(bfloat16)
    # d-major channel order: new channel j = d*19+k  <-  old channel k*4+d
    perm = np.array([4 * (j % K) + (j // K) for j in range(KD)])
    w2m = w2.reshape(KD, MID)[perm]
    w2t = np.ascontiguousarray(w2m.T).astype(bfloat16)  # [MID,KD]
    b2c = b2[perm][:, None].astype(np.float32)
    s_up = np.eye(128, k=1, dtype=np.float32).astype(bfloat16)   # out[m]=h[m-1]
    s_dn = np.eye(128, k=-1, dtype=np.float32).astype(bfloat16)  # out[m]=h[m+1]
    feats_bf = feats.astype(bfloat16)
    logits_bf = logits.astype(bfloat16)

    if _NC_CACHE is None:
        _NC_CACHE = _build()
    nc = _NC_CACHE

    in_maps = []
    for i in range(B):
        in_maps.append({
            "feats": np.ascontiguousarray(feats_bf[i]),
            "logits": np.ascontiguousarray(logits_bf[i]),
            "w1t": w1t, "bmid": bmid, "w2t": w2t, "b2": b2c,
            "sup": s_up, "sdn": s_dn,
        })

    trace = bool(os.environ.get("KTRACE"))
    res = run_bass_kernel_spmd(nc, in_maps, list(range(B)), trace=trace)
    if trace and res.exec_time_ns is not None:
        print(f"HW exec time: {res.exec_time_ns} ns")
    out = np.stack([res.results[i]["out"] for i in range(B)], axis=0)
    return out.astype(np.float32)


if __name__ == "__main__":
    rng = np.random.default_rng(0)
    ins = {
        "feats": rng.standard_normal((B, CIN, H, W), dtype=np.float32),
        "logits": rng.standard_normal((B, K, H, W), dtype=np.float32),
        "w1": rng.standard_normal((MID, CIN, 3, 3), dtype=np.float32) / 48.0,
        "gamma": rng.standard_normal(MID).astype(np.float32) * 0.1 + 1.0,
        "beta": rng.standard_normal(MID).astype(np.float32) * 0.1,
        "mean": rng.standard_normal(MID).astype(np.float32) * 0.1,
        "var": rng.random(MID).astype(np.float32) + 0.5,
        "w2": rng.standard_normal((KD, MID, 1, 1)).astype(np.float32) / 11.3,
        "b2": rng.standard_normal(KD).astype(np.float32) * 0.01,
    }
    o = kernel(**ins)
    print("kernel out", o.shape, o.dtype, np.abs(o).mean())
